# revision 1
# baseline (speedup 1.0000x reference)
"""Trainium2 Bass kernel for 3-layer GraphSAGE (mean aggr) over 8 NeuronCores.

Strategy (hardcoded for N=50000, E=800000, F=128->256->256->10):
  - Nodes sharded across 8 cores: core c owns global nodes [c*6250,(c+1)*6250),
    padded locally to 6272 = 49 groups of 128.
  - Edges partitioned by destination owner; per core, edges are sorted by local
    dst and packed into chunks of 128 edges whose dst's lie within one 128-node
    group. Chunk counts per group are equalized across cores (pad edges) so a
    single SPMD program works for all cores.
  - Gather of source-node features: indirect DMA (row gather) from a replicated
    (layer 1) or all-gathered (layers 2/3) DRAM feature table.
  - Segment mean: per chunk, selection matrix S[e, col] = (dstcol[e]==col) *
    invdeg[e] built on DVE from a host iota and per-edge scalars; aggregation
    is matmul lhsT=G (edges x F), rhs=S -> PSUM [F, nodes] accumulated over the
    group's chunks (feature-major output feeds the dense matmuls directly).
  - Layer 3 pushes the Wl matmul *before* aggregation (linearity), so only a
    [N,16] table is gathered instead of [N,256].
  - Collectives: AllGather of h1 (row-major shard) and of p3 = h2 @ W3l.T.
"""

import os
import numpy as np

_P = 128
_N, _E, _FIN, _HID, _OUT, _OUTP = 50000, 800000, 128, 256, 10, 16
_C = 8
_NL = _N // _C            # 6250
_G = (_NL + _P - 1) // _P  # 49
_NLP = _G * _P            # 6272
_NGP = _C * _NLP          # 50176
_BG1, _BG2, _BG3 = 8, 4, 24   # gather sub-chunks per indirect DMA, per layer


def _prep(x, edge_index):
    """Host-side edge partitioning. Returns per-core arrays + chunk structure."""
    src = np.asarray(edge_index[0], dtype=np.int64)
    dst = np.asarray(edge_index[1], dtype=np.int64)
    owner = dst // _NL
    dl = (dst - owner * _NL).astype(np.int64)
    # source index in the padded global layout used by h1_full / p3_full / x_pad
    srcp = ((src // _NL) * _NLP + (src % _NL)).astype(np.int64)

    per_core = []
    gdeg = np.zeros((_C, _G), dtype=np.int64)
    for c in range(_C):
        m = owner == c
        s_c, d_c = srcp[m], dl[m]
        order = np.argsort(d_c, kind="stable")
        s_c, d_c = s_c[order], d_c[order]
        deg = np.bincount(d_c, minlength=_NLP)
        gdeg[c] = deg.reshape(_G, _P).sum(1)
        per_core.append((s_c, d_c, deg))

    chunks_g = np.maximum(1, np.ceil(gdeg.max(0) / _P).astype(np.int64))  # [G]
    T = int(chunks_g.sum())
    cstart = np.concatenate([[0], np.cumsum(chunks_g)]).astype(np.int64)

    maps = []
    for c in range(_C):
        s_c, d_c, deg = per_core[c]
        invdeg = (1.0 / np.maximum(deg, 1)).astype(np.float32)
        offs = np.zeros((T, _P), np.int32)
        dcol = np.full((T, _P), -1.0, np.float32)
        ivd = np.zeros((T, _P), np.float32)
        bounds = np.searchsorted(d_c, np.arange(_G + 1) * _P, "left")
        for g in range(_G):
            lo, hi = bounds[g], bounds[g + 1]
            cnt = hi - lo
            tg = int(chunks_g[g])
            fo = np.zeros(tg * _P, np.int32)
            fo[:cnt] = s_c[lo:hi]
            fd = np.full(tg * _P, -1.0, np.float32)
            fd[:cnt] = (d_c[lo:hi] - g * _P).astype(np.float32)
            fv = np.zeros(tg * _P, np.float32)
            fv[:cnt] = invdeg[d_c[lo:hi]]
            t0 = int(cstart[g])
            offs[t0 : t0 + tg] = fo.reshape(tg, _P)
            dcol[t0 : t0 + tg] = fd.reshape(tg, _P)
            ivd[t0 : t0 + tg] = fv.reshape(tg, _P)
        xT = np.zeros((_FIN, _NLP), np.float32)
        xT[:, :_NL] = np.asarray(x[c * _NL : (c + 1) * _NL], np.float32).T
        maps.append(
            dict(
                offs=np.ascontiguousarray(offs.T),     # [128, T] int32
                dcol=np.ascontiguousarray(dcol.T),     # [128, T] f32
                ivd=np.ascontiguousarray(ivd.T),       # [128, T] f32
                xT=xT,
            )
        )
    return maps, T, chunks_g, cstart


def _build(T, chunks_g, cstart):
    """Build the SPMD Bass program. Returns (nc, input_names)."""
    import sys

    if "/opt/trn_rl_repo" not in sys.path:
        sys.path.insert(0, "/opt/trn_rl_repo")
    from concourse import bass, mybir, bacc
    import concourse.tile as tile

    f32 = mybir.dt.float32
    i32 = mybir.dt.int32
    Alu = mybir.AluOpType
    Act = mybir.ActivationFunctionType
    AxX = mybir.AxisListType.X

    nc = bacc.Bacc(
        "TRN2",
        target_bir_lowering=False,
        debug=False,
        enable_asserts=False,
        num_devices=_C,
    )

    # kernel I/O
    x_d = nc.dram_tensor("xpad", [_NGP, _FIN], f32, kind="ExternalInput")
    xT_d = nc.dram_tensor("xT", [_P, _NLP], f32, kind="ExternalInput")
    offs_d = nc.dram_tensor("offs", [_P, T], i32, kind="ExternalInput")
    dcol_d = nc.dram_tensor("dcol", [_P, T], f32, kind="ExternalInput")
    ivd_d = nc.dram_tensor("ivd", [_P, T], f32, kind="ExternalInput")
    iota_d = nc.dram_tensor("iota", [_P, _P], f32, kind="ExternalInput")
    ident_d = nc.dram_tensor("ident", [_P, _P], f32, kind="ExternalInput")
    w1l_d = nc.dram_tensor("w1lT", [_FIN, _HID], f32, kind="ExternalInput")
    w1r_d = nc.dram_tensor("w1rT", [_FIN, _HID], f32, kind="ExternalInput")
    w2l_d = nc.dram_tensor("w2lT", [_HID, _HID], f32, kind="ExternalInput")
    w2r_d = nc.dram_tensor("w2rT", [_HID, _HID], f32, kind="ExternalInput")
    w3l_d = nc.dram_tensor("w3lT", [_HID, _OUTP], f32, kind="ExternalInput")
    w3r_d = nc.dram_tensor("w3rT", [_HID, _OUTP], f32, kind="ExternalInput")
    b1_d = nc.dram_tensor("b1", [_P, 2], f32, kind="ExternalInput")
    b2_d = nc.dram_tensor("b2", [_P, 2], f32, kind="ExternalInput")
    b3_d = nc.dram_tensor("b3", [_P, _OUTP], f32, kind="ExternalInput")
    out_d = nc.dram_tensor("out", [_NLP, _OUTP], f32, kind="ExternalOutput")
    debug = os.environ.get("KDBG", "0") == "1"
    if debug:
        h1dbg_d = nc.dram_tensor("h1dbg", [_NLP, _HID], f32, kind="ExternalOutput")
        p3dbg_d = nc.dram_tensor("p3dbg", [_NLP, _OUTP], f32, kind="ExternalOutput")

    input_names = [
        "xpad", "xT", "offs", "dcol", "ivd", "iota", "ident",
        "w1lT", "w1rT", "w2lT", "w2rT", "w3lT", "w3rT", "b1", "b2", "b3",
    ]

    rg = [list(range(_C))]

    with tile.TileContext(nc) as tc:
        with (
            tc.tile_pool(name="dram", bufs=1, space="DRAM") as dp,
            tc.tile_pool(name="const", bufs=1) as cp,
            tc.tile_pool(name="gat", bufs=3) as gp,
            tc.tile_pool(name="sel", bufs=6) as sp,
            tc.tile_pool(name="work", bufs=4) as wp,
            tc.tile_pool(name="psA", bufs=2, space="PSUM") as psA,
            tc.tile_pool(name="psB", bufs=2, space="PSUM") as psB,
            tc.tile_pool(name="psT", bufs=2, space="PSUM") as psT,
        ):
            # DRAM scratch
            h1_shard = dp.tile([_NLP, _HID], f32, name="h1_shard")
            h1_full = dp.tile([_NGP, _HID], f32, name="h1_full",
                              addr_space="Shared")
            p3_shard = dp.tile([_NLP, _OUTP], f32, name="p3_shard")
            p3_full = dp.tile([_NGP, _OUTP], f32, name="p3_full",
                              addr_space="Shared")

            # ---- resident constants ----
            def load(dram, shape, dtype=f32, name=None):
                t = cp.tile(shape, dtype, name=name)
                nc.sync.dma_start(out=t[:], in_=dram.ap())
                return t

            offsT = load(offs_d, [_P, T], i32, "offsT")
            dcolT = load(dcol_d, [_P, T], f32, "dcolT")
            ivdT = load(ivd_d, [_P, T], f32, "ivdT")
            iota = load(iota_d, [_P, _P], f32, "iotaT")
            ident = load(ident_d, [_P, _P], f32, "identT")
            xT = load(xT_d, [_P, _NLP], f32, "xTt")
            w1l = load(w1l_d, [_FIN, _HID], f32, "w1lTt")
            w1r = load(w1r_d, [_FIN, _HID], f32, "w1rTt")
            w2lt, w2rt, w3lt, w3rt = [], [], [], []
            for f in range(2):
                t = cp.tile([_P, _HID], f32, name=f"w2l{f}")
                nc.sync.dma_start(out=t[:], in_=w2l_d.ap()[f * _P : (f + 1) * _P, :])
                w2lt.append(t)
                t = cp.tile([_P, _HID], f32, name=f"w2r{f}")
                nc.sync.dma_start(out=t[:], in_=w2r_d.ap()[f * _P : (f + 1) * _P, :])
                w2rt.append(t)
                t = cp.tile([_P, _OUTP], f32, name=f"w3l{f}")
                nc.sync.dma_start(out=t[:], in_=w3l_d.ap()[f * _P : (f + 1) * _P, :])
                w3lt.append(t)
                t = cp.tile([_P, _OUTP], f32, name=f"w3r{f}")
                nc.sync.dma_start(out=t[:], in_=w3r_d.ap()[f * _P : (f + 1) * _P, :])
                w3rt.append(t)
            b1 = load(b1_d, [_P, 2], f32, "b1t")
            b2 = load(b2_d, [_P, 2], f32, "b2t")
            b3 = load(b3_d, [_P, _OUTP], f32, "b3t")
            h1T = [cp.tile([_P, _NLP], f32, name=f"h1T{f}") for f in range(2)]
            h2T = [cp.tile([_P, _NLP], f32, name=f"h2T{f}") for f in range(2)]

            def make_S(t):
                S = sp.tile([_P, _P], f32, tag="S")
                nc.vector.tensor_scalar(
                    out=S[:], in0=iota[:],
                    scalar1=dcolT[:, t : t + 1], scalar2=ivdT[:, t : t + 1],
                    op0=Alu.is_equal, op1=Alu.mult,
                )
                return S

            def gather_group(g, src_ap, F, BG, tag):
                """Indirect-gather all chunks of group g; returns list of
                (chunk_index_global, sbuf_tile, free_offset)."""
                t0, tg = int(cstart[g]), int(chunks_g[g])
                out = []
                for b0 in range(0, tg, BG):
                    nb = min(BG, tg - b0)
                    gt = gp.tile([_P, nb * F], f32, tag=tag)
                    nc.gpsimd.indirect_dma_start(
                        out=gt[:],
                        out_offset=None,
                        in_=src_ap,
                        in_offset=bass.IndirectOffsetOnAxis(
                            ap=offsT[:, t0 + b0 : t0 + b0 + nb], axis=0
                        ),
                    )
                    for j in range(nb):
                        out.append((t0 + b0 + j, gt, j * F))
                return out

            # ================= Layer 1 =================
            for g in range(_G):
                t0, tg = int(cstart[g]), int(chunks_g[g])
                gl = gather_group(g, x_d.ap(), _FIN, _BG1, "g1")
                pa = psA.tile([_P, _P], f32, tag="agg")
                for k, (t, gt, fo) in enumerate(gl):
                    S = make_S(t)
                    nc.tensor.matmul(
                        out=pa[:], lhsT=gt[:, fo : fo + _FIN], rhs=S[:],
                        start=(k == 0), stop=(k == tg - 1),
                    )
                mean = wp.tile([_P, _P], f32, tag="mean")
                nc.vector.tensor_copy(out=mean[:], in_=pa[:])
                ns = slice(g * _P, (g + 1) * _P)
                for h in range(2):
                    hs = slice(h * _P, (h + 1) * _P)
                    ph = psB.tile([_P, _P], f32, tag="dense")
                    nc.tensor.matmul(out=ph[:], lhsT=w1l[:, hs], rhs=mean[:],
                                     start=True, stop=False)
                    nc.tensor.matmul(out=ph[:], lhsT=w1r[:, hs], rhs=xT[:, ns],
                                     start=False, stop=True)
                    nc.scalar.activation(out=h1T[h][:, ns], in_=ph[:],
                                         func=Act.Relu, bias=b1[:, h : h + 1])
                row = wp.tile([_P, _HID], f32, tag="row")
                for h in range(2):
                    pt = psT.tile([_P, _P], f32, tag="tr")
                    nc.tensor.transpose(out=pt[:], in_=h1T[h][:, ns],
                                        identity=ident[:])
                    nc.vector.tensor_copy(out=row[:, h * _P : (h + 1) * _P],
                                          in_=pt[:])
                nc.sync.dma_start(out=h1_shard[ns, :], in_=row[:])
                if debug:
                    nc.sync.dma_start(out=h1dbg_d.ap()[ns, :], in_=row[:])

            nc.gpsimd.collective_compute(
                "AllGather", Alu.bypass, replica_groups=rg,
                ins=[h1_shard.opt()], outs=[h1_full.opt()],
            )

            # ================= Layer 2 =================
            for g in range(_G):
                t0, tg = int(cstart[g]), int(chunks_g[g])
                gl = gather_group(g, h1_full[:], _HID, _BG2, "g2")
                pa = [psA.tile([_P, _P], f32, tag="agg", name="pa0"),
                      psT.tile([_P, _P], f32, tag="tr", name="pa1")]
                for k, (t, gt, fo) in enumerate(gl):
                    S = make_S(t)
                    for f in range(2):
                        nc.tensor.matmul(
                            out=pa[f][:],
                            lhsT=gt[:, fo + f * _P : fo + (f + 1) * _P],
                            rhs=S[:], start=(k == 0), stop=(k == tg - 1),
                        )
                mean = [wp.tile([_P, _P], f32, tag="mean", name="mean0"),
                        wp.tile([_P, _P], f32, tag="mean2", name="mean1")]
                for f in range(2):
                    nc.vector.tensor_copy(out=mean[f][:], in_=pa[f][:])
                ns = slice(g * _P, (g + 1) * _P)
                for h in range(2):
                    hs = slice(h * _P, (h + 1) * _P)
                    ph = psB.tile([_P, _P], f32, tag="dense")
                    for f in range(2):
                        nc.tensor.matmul(out=ph[:], lhsT=w2lt[f][:, hs],
                                         rhs=mean[f][:], start=(f == 0),
                                         stop=False)
                    for f in range(2):
                        nc.tensor.matmul(out=ph[:], lhsT=w2rt[f][:, hs],
                                         rhs=h1T[f][:, ns], start=False,
                                         stop=(f == 1))
                    nc.scalar.activation(out=h2T[h][:, ns], in_=ph[:],
                                         func=Act.Relu, bias=b2[:, h : h + 1])
                # p3 = h2 @ W3l.T  (row-major directly)
                pp = psA.tile([_P, _OUTP], f32, tag="p3")
                for f in range(2):
                    nc.tensor.matmul(out=pp[:], lhsT=h2T[f][:, ns],
                                     rhs=w3lt[f][:], start=(f == 0),
                                     stop=(f == 1))
                p3row = wp.tile([_P, _OUTP], f32, tag="p3row")
                nc.vector.tensor_copy(out=p3row[:], in_=pp[:])
                nc.sync.dma_start(out=p3_shard[ns, :], in_=p3row[:])
                if debug:
                    nc.sync.dma_start(out=p3dbg_d.ap()[ns, :], in_=p3row[:])

            nc.gpsimd.collective_compute(
                "AllGather", Alu.bypass, replica_groups=rg,
                ins=[p3_shard.opt()], outs=[p3_full.opt()],
            )

            # ================= Layer 3 + log_softmax =================
            for g in range(_G):
                t0, tg = int(cstart[g]), int(chunks_g[g])
                gl = gather_group(g, p3_full[:], _OUTP, _BG3, "g3")
                po = psB.tile([_P, _OUTP], f32, tag="dense")
                for k, (t, gt, fo) in enumerate(gl):
                    S = make_S(t)
                    nc.tensor.matmul(out=po[:], lhsT=S[:],
                                     rhs=gt[:, fo : fo + _OUTP],
                                     start=(k == 0), stop=False)
                ns = slice(g * _P, (g + 1) * _P)
                for f in range(2):
                    nc.tensor.matmul(out=po[:], lhsT=h2T[f][:, ns],
                                     rhs=w3rt[f][:], start=False, stop=(f == 1))
                z = wp.tile([_P, _OUTP], f32, tag="z")
                nc.vector.tensor_tensor(out=z[:], in0=po[:], in1=b3[:],
                                        op=Alu.add)
                mx = sp.tile([_P, 1], f32, tag="mx")
                nc.vector.reduce_max(mx[:], z[:], axis=AxX)
                zc = wp.tile([_P, _OUTP], f32, tag="zc")
                nc.vector.tensor_scalar(out=zc[:], in0=z[:], scalar1=mx[:],
                                        scalar2=None, op0=Alu.subtract)
                ez = wp.tile([_P, _OUTP], f32, tag="ez")
                nc.scalar.activation(out=ez[:], in_=zc[:], func=Act.Exp)
                sm = sp.tile([_P, 1], f32, tag="sm")
                nc.vector.reduce_sum(sm[:], ez[:], axis=AxX)
                lg = sp.tile([_P, 1], f32, tag="lg")
                nc.scalar.activation(out=lg[:], in_=sm[:], func=Act.Ln)
                res = wp.tile([_P, _OUTP], f32, tag="res")
                nc.vector.tensor_scalar(out=res[:], in0=zc[:], scalar1=lg[:],
                                        scalar2=None, op0=Alu.subtract)
                nc.sync.dma_start(out=out_d.ap()[ns, :], in_=res[:])

    nc.compile()
    return nc, input_names


def _run(inputs, trace=False, tmpdir=None):
    import sys

    if "/opt/trn_rl_repo" not in sys.path:
        sys.path.insert(0, "/opt/trn_rl_repo")
    from concourse import bass_utils

    x = np.asarray(inputs["x"], np.float32)
    maps, T, chunks_g, cstart = _prep(x, inputs["edge_index"])

    x_pad = np.zeros((_NGP, _FIN), np.float32)
    for c in range(_C):
        x_pad[c * _NLP : c * _NLP + _NL] = x[c * _NL : (c + 1) * _NL]

    iota = np.tile(np.arange(_P, dtype=np.float32), (_P, 1))
    ident = np.eye(_P, dtype=np.float32)
    w1lT = np.ascontiguousarray(np.asarray(inputs["W1l"], np.float32).T)
    w1rT = np.ascontiguousarray(np.asarray(inputs["W1r"], np.float32).T)
    w2lT = np.ascontiguousarray(np.asarray(inputs["W2l"], np.float32).T)
    w2rT = np.ascontiguousarray(np.asarray(inputs["W2r"], np.float32).T)
    w3lT = np.zeros((_HID, _OUTP), np.float32)
    w3lT[:, :_OUT] = np.asarray(inputs["W3l"], np.float32).T
    w3rT = np.zeros((_HID, _OUTP), np.float32)
    w3rT[:, :_OUT] = np.asarray(inputs["W3r"], np.float32).T
    b1 = np.ascontiguousarray(
        np.asarray(inputs["b1l"], np.float32).reshape(2, _P).T)
    b2 = np.ascontiguousarray(
        np.asarray(inputs["b2l"], np.float32).reshape(2, _P).T)
    b3 = np.full((_P, _OUTP), -1e9, np.float32)
    b3[:, :_OUT] = np.asarray(inputs["b3l"], np.float32)[None, :]

    shared = dict(
        xpad=x_pad, iota=iota, ident=ident,
        w1lT=w1lT, w1rT=w1rT, w2lT=w2lT, w2rT=w2rT, w3lT=w3lT, w3rT=w3rT,
        b1=b1, b2=b2, b3=b3,
    )
    in_maps = []
    for c in range(_C):
        m = dict(shared)
        m["xT"] = maps[c]["xT"]
        m["offs"] = maps[c]["offs"]
        m["dcol"] = maps[c]["dcol"]
        m["ivd"] = maps[c]["ivd"]
        in_maps.append(m)

    nc, input_names = _build(T, chunks_g, cstart)

    res = bass_utils.run_bass_kernel_spmd(
        nc, in_maps, core_ids=list(range(_C)), trace=trace, tmpdir=tmpdir,
    )
    outs = res.results
    y = np.concatenate(
        [np.asarray(outs[c]["out"])[:_NL, :_OUT] for c in range(_C)], axis=0
    ).astype(np.float32)
    return y, res


def kernel(**inputs):
    y, _ = _run(inputs, trace=False)
    return y



# revision 8
# speedup vs baseline: 2.0777x; 2.0777x over previous
"""Trainium2 Bass kernel for 3-layer GraphSAGE (mean aggr) over 8 NeuronCores.

Strategy (hardcoded for N=50000, E=800000, F=128->256->256->10):
  - Nodes sharded across 8 cores: core c owns global nodes [c*6250,(c+1)*6250),
    padded locally to 6272 = 49 groups of 128.
  - Edges partitioned by destination owner; per core, edges are sorted by local
    dst and packed into chunks of 128 edges whose dst's lie within one 128-node
    group. Chunk counts per group are equalized across cores (pad edges) so a
    single SPMD program works for all cores.
  - fp16 datapath: gathered features, S selection matrices, weights and hidden
    activations are fp16 (PSUM accumulation fp32). This quadruples PE matmul
    throughput (1 cycle/row vs 4 for fp32) and halves gather/collective bytes.
  - Gather of source-node features: indirect DMA row gather, batched ~32 chunks
    per SWDGE call, from a replicated (layer 1) or all-gathered (layers 2/3)
    DRAM feature table.
  - Segment mean (L1/L2): S[e, col] = (dstcol[e]==col) * invdeg[e] built on DVE
    from an iota + per-edge scalars; aggregation is matmul lhsT=G (edges x F),
    rhs=S -> PSUM [F, nodes group] accumulated over the group's chunks.
  - Layer 1 computes h1 in BOTH orientations directly from PSUM (row-major for
    the AllGather/gather table via lhsT=mean trick; feature-major for L2's Wr
    term) -- no PE transposes.
  - Layer 3 pushes the Wl matmul before aggregation (linearity) so only a
    [N,16] fp16 table is gathered; aggregation is matmul lhsT=S, rhs=gathered
    rows directly in node-major orientation. log_softmax epilogue is batched
    (one Exp over all groups, one Ln).
  - Collectives: AllGather of h1 is split into 4 node-range chunks issued as
    soon as their layer-1 groups finish (overlaps the collective with compute);
    h1_full uses a chunk-major layout [q][core][rows][HID] so each chunked
    AllGather's output is contiguous. p3 AllGather split in 2 halves likewise.
"""

import os
import numpy as np

_P = 128
_N, _E, _FIN, _HID, _OUT, _OUTP = 50000, 800000, 128, 256, 10, 16
_C = 8
_NL = _N // _C            # 6250
_G = (_NL + _P - 1) // _P  # 49
_NLP = _G * _P            # 6272
_NGP = _C * _NLP          # 50176
_BG1, _BG2, _BG3 = 32, 16, 64   # gather chunks per indirect DMA, per layer

# AllGather chunking: groups per chunk (sum = 49)
_QG = [13, 12, 12, 12]          # h1 (4 chunks)
_HG = [25, 24]                  # p3 (2 chunks)
_QSTART = np.concatenate([[0], np.cumsum(_QG)]) * _P   # local row starts
_HSTART = np.concatenate([[0], np.cumsum(_HG)]) * _P


def _q_of(i):
    """AllGather chunk (quarter) of local padded row i, vectorized."""
    i = np.asarray(i)
    q = np.searchsorted(_QSTART[1:], i, side="right")
    return q


def _h_of(i):
    i = np.asarray(i)
    return np.searchsorted(_HSTART[1:], i, side="right")


def _prep(x, edge_index):
    """Host-side edge partitioning. Returns per-core arrays + chunk structure."""
    src = np.asarray(edge_index[0], dtype=np.int64)
    dst = np.asarray(edge_index[1], dtype=np.int64)
    owner = dst // _NL
    dl = (dst - owner * _NL).astype(np.int64)
    sc = (src // _NL).astype(np.int64)   # source core
    si = (src % _NL).astype(np.int64)    # source local row (always < NL)

    # layer-1 source rows: x_pad is core-major [c*NLP + i]
    srcp1 = sc * _NLP + si
    # layer-2 source rows: h1_full is quarter-chunk-major
    q = _q_of(si)
    qlen = np.asarray(_QG) * _P
    srcp2 = 8 * _QSTART[q] + sc * qlen[q] + (si - _QSTART[q])
    # layer-3 source rows: p3_full is half-chunk-major
    h = _h_of(si)
    hlen = np.asarray(_HG) * _P
    srcp3 = 8 * _HSTART[h] + sc * hlen[h] + (si - _HSTART[h])

    per_core = []
    gdeg = np.zeros((_C, _G), dtype=np.int64)
    for c in range(_C):
        m = owner == c
        d_c = dl[m]
        order = np.argsort(d_c, kind="stable")
        d_c = d_c[order]
        s1, s2, s3 = srcp1[m][order], srcp2[m][order], srcp3[m][order]
        deg = np.bincount(d_c, minlength=_NLP)
        gdeg[c] = deg.reshape(_G, _P).sum(1)
        per_core.append((d_c, s1, s2, s3, deg))

    chunks_g = np.maximum(1, np.ceil(gdeg.max(0) / _P).astype(np.int64))  # [G]
    T = int(chunks_g.sum())
    cstart = np.concatenate([[0], np.cumsum(chunks_g)]).astype(np.int64)

    maps = []
    for c in range(_C):
        d_c, s1, s2, s3, deg = per_core[c]
        invdeg = (1.0 / np.maximum(deg, 1)).astype(np.float32)
        offs1 = np.zeros((T, _P), np.int32)
        offs2 = np.zeros((T, _P), np.int32)
        offs3 = np.zeros((T, _P), np.int32)
        dcol = np.full((T, _P), -1.0, np.float32)
        ivd = np.zeros((T, _P), np.float32)
        bounds = np.searchsorted(d_c, np.arange(_G + 1) * _P, "left")
        for g in range(_G):
            lo, hi = bounds[g], bounds[g + 1]
            cnt = hi - lo
            tg = int(chunks_g[g])
            t0 = int(cstart[g])

            def fill(dstarr, vals, fillv, dtype):
                fo = np.full(tg * _P, fillv, dtype)
                fo[:cnt] = vals
                dstarr[t0 : t0 + tg] = fo.reshape(tg, _P)

            fill(offs1, s1[lo:hi], 0, np.int32)
            fill(offs2, s2[lo:hi], 0, np.int32)
            fill(offs3, s3[lo:hi], 0, np.int32)
            fill(dcol, (d_c[lo:hi] - g * _P).astype(np.float32), -1.0, np.float32)
            fill(ivd, invdeg[d_c[lo:hi]], 0.0, np.float32)

        xT = np.zeros((_FIN, _NLP), np.float16)
        xT[:, :_NL] = np.asarray(x[c * _NL : (c + 1) * _NL], np.float32).T
        maps.append(
            dict(
                offs1=np.ascontiguousarray(offs1.T),
                offs2=np.ascontiguousarray(offs2.T),
                offs3=np.ascontiguousarray(offs3.T),
                dcol=np.ascontiguousarray(dcol.T),
                ivd=np.ascontiguousarray(ivd.T),
                xT=xT,
            )
        )
    return maps, T, chunks_g, cstart


def _build(T, chunks_g, cstart):
    """Build the SPMD Bass program. Returns (nc, input_names)."""
    import sys

    if "/opt/trn_rl_repo" not in sys.path:
        sys.path.insert(0, "/opt/trn_rl_repo")
    from concourse import bass, mybir, bacc
    import concourse.tile as tile

    f32 = mybir.dt.float32
    f16 = mybir.dt.float16
    i32 = mybir.dt.int32
    Alu = mybir.AluOpType
    Act = mybir.ActivationFunctionType
    AxX = mybir.AxisListType.X

    nc = bacc.Bacc(
        "TRN2",
        target_bir_lowering=False,
        debug=False,
        enable_asserts=False,
        num_devices=_C,
    )

    # kernel I/O
    x_d = nc.dram_tensor("xpad", [_NGP, _FIN], f16, kind="ExternalInput")
    xT_d = nc.dram_tensor("xT", [_P, _NLP], f16, kind="ExternalInput")
    offs1_d = nc.dram_tensor("offs1", [_P, T], i32, kind="ExternalInput")
    offs2_d = nc.dram_tensor("offs2", [_P, T], i32, kind="ExternalInput")
    offs3_d = nc.dram_tensor("offs3", [_P, T], i32, kind="ExternalInput")
    dcol_d = nc.dram_tensor("dcol", [_P, T], f32, kind="ExternalInput")
    ivd_d = nc.dram_tensor("ivd", [_P, T], f32, kind="ExternalInput")
    iota_d = nc.dram_tensor("iota", [_P, _P], f16, kind="ExternalInput")
    w1l_d = nc.dram_tensor("w1lT", [_FIN, _HID], f16, kind="ExternalInput")
    w1r_d = nc.dram_tensor("w1rT", [_FIN, _HID], f16, kind="ExternalInput")
    w2l_d = nc.dram_tensor("w2lT", [_HID, _HID], f16, kind="ExternalInput")
    w2r_d = nc.dram_tensor("w2rT", [_HID, _HID], f16, kind="ExternalInput")
    w3l_d = nc.dram_tensor("w3lT", [_HID, _OUTP], f16, kind="ExternalInput")
    w3r_d = nc.dram_tensor("w3rT", [_HID, _OUTP], f16, kind="ExternalInput")
    b1_d = nc.dram_tensor("b1", [_P, 2], f32, kind="ExternalInput")
    b2_d = nc.dram_tensor("b2", [_P, 2], f32, kind="ExternalInput")
    b1row_d = nc.dram_tensor("b1row", [1, _HID], f16, kind="ExternalInput")
    b3row_d = nc.dram_tensor("b3row", [1, _OUTP], f16, kind="ExternalInput")
    out_d = nc.dram_tensor("out", [_NLP, _OUTP], f32, kind="ExternalOutput")
    debug = os.environ.get("KDBG", "0") == "1"
    if debug:
        h1dbg_d = nc.dram_tensor("h1dbg", [_NLP, _HID], f16, kind="ExternalOutput")
        p3dbg_d = nc.dram_tensor("p3dbg", [_NLP, _OUTP], f16, kind="ExternalOutput")

    input_names = [
        "xpad", "xT", "offs1", "offs2", "offs3", "dcol", "ivd",
        "iota", "w1lT", "w1rT", "w2lT", "w2rT", "w3lT", "w3rT",
        "b1", "b2", "b1row", "b3row",
    ]

    rg = [list(range(_C))]

    with tile.TileContext(nc) as tc:
        with (
            tc.tile_pool(name="dram", bufs=1, space="DRAM") as dp,
            tc.tile_pool(name="const", bufs=1) as cp,
            tc.tile_pool(name="g1", bufs=3) as gp1,
            tc.tile_pool(name="g2", bufs=3) as gp2,
            tc.tile_pool(name="g3", bufs=3) as gp3,
            tc.tile_pool(name="sel", bufs=8) as sp,
            tc.tile_pool(name="work", bufs=4) as wp,
            tc.tile_pool(name="psA", bufs=2, space="PSUM") as psA,
            tc.tile_pool(name="psB", bufs=2, space="PSUM") as psB,
            tc.tile_pool(name="psR", bufs=1, space="PSUM") as psR,
            tc.tile_pool(name="psP", bufs=1, space="PSUM") as psP,
        ):
            # DRAM scratch
            h1_shard = dp.tile([_NLP, _HID], f16, name="h1_shard")
            h1_full = dp.tile([_NGP, _HID], f16, name="h1_full")
            p3_shard = dp.tile([_NLP, _OUTP], f16, name="p3_shard")
            p3_full = dp.tile([_NGP, _OUTP], f16, name="p3_full")

            # ---- resident constants ----
            def load(dram, shape, dtype, name):
                t = cp.tile(shape, dtype, name=name)
                nc.sync.dma_start(out=t[:], in_=dram.ap())
                return t

            offs1 = load(offs1_d, [_P, T], i32, "offs1t")
            offs2 = load(offs2_d, [_P, T], i32, "offs2t")
            offs3 = load(offs3_d, [_P, T], i32, "offs3t")
            dcolT = load(dcol_d, [_P, T], f32, "dcolT")
            ivdT = load(ivd_d, [_P, T], f32, "ivdT")
            iota = load(iota_d, [_P, _P], f16, "iotaT")
            xT = load(xT_d, [_P, _NLP], f16, "xTt")
            w1l = load(w1l_d, [_FIN, _HID], f16, "w1lTt")
            w1r = load(w1r_d, [_FIN, _HID], f16, "w1rTt")
            w2lt, w2rt, w3lt, w3rt = [], [], [], []
            for f in range(2):
                t = cp.tile([_P, _HID], f16, name=f"w2l{f}")
                nc.sync.dma_start(out=t[:], in_=w2l_d.ap()[f * _P : (f + 1) * _P, :])
                w2lt.append(t)
                t = cp.tile([_P, _HID], f16, name=f"w2r{f}")
                nc.sync.dma_start(out=t[:], in_=w2r_d.ap()[f * _P : (f + 1) * _P, :])
                w2rt.append(t)
                t = cp.tile([_P, _OUTP], f16, name=f"w3l{f}")
                nc.sync.dma_start(out=t[:], in_=w3l_d.ap()[f * _P : (f + 1) * _P, :])
                w3lt.append(t)
                t = cp.tile([_P, _OUTP], f16, name=f"w3r{f}")
                nc.sync.dma_start(out=t[:], in_=w3r_d.ap()[f * _P : (f + 1) * _P, :])
                w3rt.append(t)
            b1 = load(b1_d, [_P, 2], f32, "b1t")
            b2 = load(b2_d, [_P, 2], f32, "b2t")
            b1row = load(b1row_d, [1, _HID], f16, "b1rowt")
            b3row = load(b3row_d, [1, _OUTP], f16, "b3rowt")
            ones1 = cp.tile([1, _P], f16, name="ones1")
            nc.vector.memset(ones1[:], 1.0)
            h1T = [cp.tile([_P, _NLP], f16, name=f"h1T{f}") for f in range(2)]
            h2T = [cp.tile([_P, _NLP], f16, name=f"h2T{f}") for f in range(2)]
            zbuf = cp.tile([_P, _G, _OUTP], f32, name="zbuf")
            ezbuf = cp.tile([_P, _G, _OUTP], f32, name="ezbuf")
            mxbuf = cp.tile([_P, _G], f32, name="mxbuf")
            smbuf = cp.tile([_P, _G], f32, name="smbuf")
            lgbuf = cp.tile([_P, _G], f32, name="lgbuf")

            def make_S(t):
                S = sp.tile([_P, _P], f16, tag="S")
                nc.vector.tensor_scalar(
                    out=S[:], in0=iota[:],
                    scalar1=dcolT[:, t : t + 1], scalar2=ivdT[:, t : t + 1],
                    op0=Alu.is_equal, op1=Alu.mult,
                )
                return S

            def make_get(pool, offs_tile, src_ap, F, BG, tag, dt):
                issued = {}

                def get(t):
                    bi = t // BG
                    if bi not in issued:
                        b0 = bi * BG
                        nb = min(BG, T - b0)
                        gt = pool.tile([_P, nb * F], dt, tag=tag)
                        nc.gpsimd.indirect_dma_start(
                            out=gt[:],
                            out_offset=None,
                            in_=src_ap,
                            in_offset=bass.IndirectOffsetOnAxis(
                                ap=offs_tile[:, b0 : b0 + nb], axis=0
                            ),
                        )
                        issued[bi] = gt
                    return issued[bi], (t - bi * BG) * F

                return get

            # ================= Layer 1 =================
            get1 = make_get(gp1, offs1, x_d.ap(), _FIN, _BG1, "g1", f16)
            q_at = {int(np.cumsum(_QG)[q]) - 1: q for q in range(len(_QG))}
            for g in range(_G):
                t0, tg = int(cstart[g]), int(chunks_g[g])
                pa = psA.tile([_P, _P], f32, tag="agg0")
                for k in range(tg):
                    t = t0 + k
                    gt, fo = get1(t)
                    S = make_S(t)
                    nc.tensor.matmul(
                        out=pa[:], lhsT=gt[:, fo : fo + _FIN], rhs=S[:],
                        start=(k == 0), stop=(k == tg - 1),
                    )
                mean = wp.tile([_P, _P], f16, tag="mean1")
                nc.scalar.copy(out=mean[:], in_=pa[:])
                ns = slice(g * _P, (g + 1) * _P)
                # feature-major h1T (for L2's Wr term)
                for h in range(2):
                    hs = slice(h * _P, (h + 1) * _P)
                    ph = psB.tile([_P, _P], f32, tag="dense")
                    nc.tensor.matmul(out=ph[:], lhsT=w1l[:, hs], rhs=mean[:],
                                     start=True, stop=False)
                    nc.tensor.matmul(out=ph[:], lhsT=w1r[:, hs], rhs=xT[:, ns],
                                     start=False, stop=True)
                    nc.scalar.activation(out=h1T[h][:, ns], in_=ph[:],
                                         func=Act.Relu, bias=b1[:, h : h + 1])
                # row-major h1 (for the AllGather + L2 gather table)
                pr = psR.tile([_P, _HID], f32, tag="row")
                nc.tensor.matmul(out=pr[:], lhsT=mean[:], rhs=w1l[:],
                                 start=True, stop=False)
                nc.tensor.matmul(out=pr[:], lhsT=xT[:, ns], rhs=w1r[:],
                                 start=False, stop=False)
                nc.tensor.matmul(out=pr[:], lhsT=ones1[:], rhs=b1row[:],
                                 start=False, stop=True)
                row = wp.tile([_P, _HID], f16, tag="row")
                nc.scalar.activation(out=row[:], in_=pr[:], func=Act.Relu)
                nc.sync.dma_start(out=h1_shard[ns, :], in_=row[:])
                if debug:
                    nc.sync.dma_start(out=h1dbg_d.ap()[ns, :], in_=row[:])
                if g in q_at:
                    q = q_at[g]
                    qs, qe = int(_QSTART[q]), int(_QSTART[q + 1])
                    nc.gpsimd.collective_compute(
                        "AllGather", Alu.bypass, replica_groups=rg,
                        ins=[h1_shard[qs:qe, :]],
                        outs=[h1_full[8 * qs : 8 * qe, :]],
                    )

            # ================= Layer 2 =================
            get2 = make_get(gp2, offs2, h1_full[:], _HID, _BG2, "g2", f16)
            h_at = {int(np.cumsum(_HG)[h]) - 1: h for h in range(len(_HG))}
            for g in range(_G):
                t0, tg = int(cstart[g]), int(chunks_g[g])
                pa = [psA.tile([_P, _P], f32, tag="agg0", name="pa0"),
                      psA.tile([_P, _P], f32, tag="agg1", name="pa1")]
                for k in range(tg):
                    t = t0 + k
                    gt, fo = get2(t)
                    S = make_S(t)
                    for f in range(2):
                        nc.tensor.matmul(
                            out=pa[f][:],
                            lhsT=gt[:, fo + f * _P : fo + (f + 1) * _P],
                            rhs=S[:], start=(k == 0), stop=(k == tg - 1),
                        )
                mean = [wp.tile([_P, _P], f16, tag="mean20", name="mean0"),
                        wp.tile([_P, _P], f16, tag="mean21", name="mean1")]
                for f in range(2):
                    nc.scalar.copy(out=mean[f][:], in_=pa[f][:])
                ns = slice(g * _P, (g + 1) * _P)
                for h in range(2):
                    hs = slice(h * _P, (h + 1) * _P)
                    ph = psB.tile([_P, _P], f32, tag="dense")
                    for f in range(2):
                        nc.tensor.matmul(out=ph[:], lhsT=w2lt[f][:, hs],
                                         rhs=mean[f][:], start=(f == 0),
                                         stop=False)
                    for f in range(2):
                        nc.tensor.matmul(out=ph[:], lhsT=w2rt[f][:, hs],
                                         rhs=h1T[f][:, ns], start=False,
                                         stop=(f == 1))
                    nc.scalar.activation(out=h2T[h][:, ns], in_=ph[:],
                                         func=Act.Relu, bias=b2[:, h : h + 1])
                # p3 = h2 @ W3l.T  (row-major directly)
                pp = psP.tile([_P, _OUTP], f32, tag="pp")
                for f in range(2):
                    nc.tensor.matmul(out=pp[:], lhsT=h2T[f][:, ns],
                                     rhs=w3lt[f][:], start=(f == 0),
                                     stop=(f == 1))
                p3row = wp.tile([_P, _OUTP], f16, tag="p3row")
                nc.scalar.copy(out=p3row[:], in_=pp[:])
                nc.sync.dma_start(out=p3_shard[ns, :], in_=p3row[:])
                if debug:
                    nc.sync.dma_start(out=p3dbg_d.ap()[ns, :], in_=p3row[:])
                if g in h_at:
                    h = h_at[g]
                    hs0, he = int(_HSTART[h]), int(_HSTART[h + 1])
                    nc.gpsimd.collective_compute(
                        "AllGather", Alu.bypass, replica_groups=rg,
                        ins=[p3_shard[hs0:he, :]],
                        outs=[p3_full[8 * hs0 : 8 * he, :]],
                    )

            # ================= Layer 3 + log_softmax =================
            get3 = make_get(gp3, offs3, p3_full[:], _OUTP, _BG3, "g3", f16)
            for g in range(_G):
                t0, tg = int(cstart[g]), int(chunks_g[g])
                ns = slice(g * _P, (g + 1) * _P)
                po = psP.tile([_P, _OUTP], f32, tag="pp")
                for k in range(tg):
                    t = t0 + k
                    gt, fo = get3(t)
                    S = make_S(t)
                    nc.tensor.matmul(out=po[:], lhsT=S[:],
                                     rhs=gt[:, fo : fo + _OUTP],
                                     start=(k == 0), stop=False)
                for f in range(2):
                    nc.tensor.matmul(out=po[:], lhsT=h2T[f][:, ns],
                                     rhs=w3rt[f][:], start=False, stop=False)
                nc.tensor.matmul(out=po[:], lhsT=ones1[:], rhs=b3row[:],
                                 start=False, stop=True)
                nc.vector.reduce_max(mxbuf[:, g : g + 1], po[:, 0:_OUT],
                                     axis=AxX)
                nc.vector.tensor_scalar(out=zbuf[:, g, :], in0=po[:],
                                        scalar1=mxbuf[:, g : g + 1],
                                        scalar2=None, op0=Alu.subtract)
            nc.scalar.activation(out=ezbuf[:], in_=zbuf[:], func=Act.Exp)
            nc.vector.reduce_sum(smbuf[:], ezbuf[:, :, 0:_OUT], axis=AxX)
            nc.scalar.activation(out=lgbuf[:], in_=smbuf[:], func=Act.Ln)
            for g in range(_G):
                ns = slice(g * _P, (g + 1) * _P)
                res = wp.tile([_P, _OUTP], f32, tag="res")
                nc.vector.tensor_scalar(out=res[:], in0=zbuf[:, g, :],
                                        scalar1=lgbuf[:, g : g + 1],
                                        scalar2=None, op0=Alu.subtract)
                nc.sync.dma_start(out=out_d.ap()[ns, :], in_=res[:])

    nc.compile()
    return nc, input_names


def _run(inputs, trace=False, tmpdir=None):
    import sys

    if "/opt/trn_rl_repo" not in sys.path:
        sys.path.insert(0, "/opt/trn_rl_repo")
    from concourse import bass_utils

    x = np.asarray(inputs["x"], np.float32)
    maps, T, chunks_g, cstart = _prep(x, inputs["edge_index"])

    x_pad = np.zeros((_NGP, _FIN), np.float16)
    for c in range(_C):
        x_pad[c * _NLP : c * _NLP + _NL] = x[c * _NL : (c + 1) * _NL]

    iota = np.tile(np.arange(_P, dtype=np.float16), (_P, 1))
    w1lT = np.ascontiguousarray(np.asarray(inputs["W1l"], np.float32).T).astype(np.float16)
    w1rT = np.ascontiguousarray(np.asarray(inputs["W1r"], np.float32).T).astype(np.float16)
    w2lT = np.ascontiguousarray(np.asarray(inputs["W2l"], np.float32).T).astype(np.float16)
    w2rT = np.ascontiguousarray(np.asarray(inputs["W2r"], np.float32).T).astype(np.float16)
    w3lT = np.zeros((_HID, _OUTP), np.float16)
    w3lT[:, :_OUT] = np.asarray(inputs["W3l"], np.float32).T
    w3rT = np.zeros((_HID, _OUTP), np.float16)
    w3rT[:, :_OUT] = np.asarray(inputs["W3r"], np.float32).T
    b1 = np.ascontiguousarray(
        np.asarray(inputs["b1l"], np.float32).reshape(2, _P).T)
    b2 = np.ascontiguousarray(
        np.asarray(inputs["b2l"], np.float32).reshape(2, _P).T)
    b1row = np.asarray(inputs["b1l"], np.float32).reshape(1, _HID).astype(np.float16)
    b3row = np.zeros((1, _OUTP), np.float16)
    b3row[0, :_OUT] = np.asarray(inputs["b3l"], np.float32)

    shared = dict(
        xpad=x_pad, iota=iota,
        w1lT=w1lT, w1rT=w1rT, w2lT=w2lT, w2rT=w2rT, w3lT=w3lT, w3rT=w3rT,
        b1=b1, b2=b2, b1row=b1row, b3row=b3row,
    )
    in_maps = []
    for c in range(_C):
        m = dict(shared)
        for k in ("xT", "offs1", "offs2", "offs3", "dcol", "ivd"):
            m[k] = maps[c][k]
        in_maps.append(m)

    nc, input_names = _build(T, chunks_g, cstart)

    res = bass_utils.run_bass_kernel_spmd(
        nc, in_maps, core_ids=list(range(_C)), trace=trace, tmpdir=tmpdir,
    )
    outs = res.results
    y = np.concatenate(
        [np.asarray(outs[c]["out"])[:_NL, :_OUT] for c in range(_C)], axis=0
    ).astype(np.float32)
    return y, res


def kernel(**inputs):
    y, _ = _run(inputs, trace=False)
    return y
